# revision 43
# baseline (speedup 1.0000x reference)
"""Distributed Trainium2 attention kernel (8 NeuronCores), v2.

Reference computation (dense transformer attention block, prefill):
    q/k/v = x @ w{q,k,v}.T ; RoPE(q, k) ; GQA expand ; softmax(q k^T * scale + mask) v ; @ wo.T

Sharding: data-parallel over (batch x sequence): core i owns 512 tokens of
batch i//4.  Each core computes its k/v shard (contiguous 512-token block),
K and V are AllGathered in ONE collective within each batch's group of 4
cores (bf16), then each core runs attention for its 512 query tokens and
its rows of the output projection.

Query assignment (causal): core j of a batch group takes the 256-token
blocks {j, 7-j}.  Key chunks 0..7 are live for both halves (full-width
ops); chunks 8..15 only for the late half (half-width, packed two chunks
per exps row).

v2 schedule notes:
  - All input DMAs are batched (few large issues) and ordered so K proj
    starts ~5us in and the combined K+V AllGather triggers right after
    V proj, overlapping Q projection.
  - exps stored packed [P, 12, 512]; mask applied only where it can be
    non-trivial for some core: early halves of chunks 0..7 + late chunks.
  - softmax fold tree split between DVE and GpSimd; head loop software-
    pipelined one head deep so the PE never waits on the tree.
  - wo streamed once (not twice) in [P,2,512] tiles on the sync queue;
    final outputs written from the scalar engine's DMA queue.
"""

import math
import sys
import types

import numpy as np
import ml_dtypes

# ---------------------------------------------------------------------------
# antenv.axon_hooks shim: the container image's antenv package lacks
# axon_hooks; bass_utils imports it when BASS_TRACE is set.
if "antenv.axon_hooks" not in sys.modules:
    _hooks = types.ModuleType("antenv.axon_hooks")
    _hooks._hook = None
    _hooks.set_axon_ntff_profile_hook = lambda h: setattr(_hooks, "_hook", h)
    _hooks.get_axon_ntff_profile_hook = lambda: _hooks._hook
    sys.modules["antenv.axon_hooks"] = _hooks
    try:
        import antenv

        antenv.axon_hooks = _hooks
        from trn_agent_boot.trn_boot import _ntff_profile_via_ctypes

        _hooks.set_axon_ntff_profile_hook(
            _ntff_profile_via_ctypes("/opt/axon/libaxon_pjrt.so")
        )
    except Exception:
        pass

import concourse.bass as bass
import concourse.bacc as bacc
import concourse.mybir as mybir
import concourse.tile as tile
from concourse.bass_utils import run_bass_kernel_spmd

# Problem constants (hardcoded per spec nn_Attention_73040213836414).
DIM = 2048
N_HEADS = 16
N_KV_HEADS = 4
HEAD_DIM = 128
BATCH = 2
SEQLEN = 2048
N_CORES = 8
GROUPS = [[0, 1, 2, 3], [4, 5, 6, 7]]

P = 128
T = 512  # tokens per core
HT = T // 2  # 256, causal half-block
CK = DIM // P  # 16 contraction chunks
UC = SEQLEN // P  # 16 key chunks
KVW = N_KV_HEADS * HEAD_DIM  # 512

F32 = mybir.dt.float32
BF16 = mybir.dt.bfloat16
ADD = mybir.AluOpType.add
MULT = mybir.AluOpType.mult
BF = ml_dtypes.bfloat16


def build_graph(causal):
    nc = bacc.Bacc(
        "TRN2",
        target_bir_lowering=False,
        debug=False,
        enable_asserts=False,
        num_devices=N_CORES,
    )
    x_q = nc.dram_tensor("x_q", [DIM, T], BF16, kind="ExternalInput").ap()
    x_kv = nc.dram_tensor("x_kv", [DIM, T], BF16, kind="ExternalInput").ap()
    wq_t = nc.dram_tensor("wq_t", [DIM, N_HEADS * HEAD_DIM], BF16, kind="ExternalInput").ap()
    wk_t = nc.dram_tensor("wk_t", [DIM, KVW], BF16, kind="ExternalInput").ap()
    wv_t = nc.dram_tensor("wv_t", [DIM, KVW], BF16, kind="ExternalInput").ap()
    wo_t = nc.dram_tensor("wo_t", [DIM, DIM], BF16, kind="ExternalInput").ap()
    cosq = nc.dram_tensor("cosq", [P, T], F32, kind="ExternalInput").ap()
    sinq = nc.dram_tensor("sinq", [P, T], F32, kind="ExternalInput").ap()
    cosk = nc.dram_tensor("cosk", [P, T], F32, kind="ExternalInput").ap()
    sink = nc.dram_tensor("sink", [P, T], F32, kind="ExternalInput").ap()
    nexp = 12 if causal else UC
    emask_p = nc.dram_tensor("emask_p", [nexp * P, T], BF16, kind="ExternalInput").ap()
    out_e = nc.dram_tensor("out", [T, DIM], F32, kind="ExternalOutput").ap()

    with tile.TileContext(nc) as tc:
        _body(tc, nc, x_q, x_kv, wq_t, wk_t, wv_t, wo_t,
              cosq, sinq, cosk, sink, emask_p, out_e, causal)
    nc.compile()
    return nc


def _rope(nc, pool_rot, pool_tmp, psum_ap, cos_sb, sin_sb, out_ap):
    """out = psum*cos + rot_half(psum)*sin_signed, cast to out dtype."""
    rot = pool_rot.tile([P, T], F32, tag="rot")
    nc.vector.tensor_tensor(rot[0:64, :], psum_ap[64:128, :], sin_sb[0:64, :], MULT)
    nc.vector.tensor_tensor(rot[64:128, :], psum_ap[0:64, :], sin_sb[64:128, :], MULT)
    qc = pool_tmp.tile([P, T], F32, tag="tmp")
    nc.vector.tensor_tensor(qc[:], psum_ap[:], cos_sb[:], MULT)
    nc.vector.tensor_tensor(out_ap, qc[:], rot[:], ADD)


def _body(tc, nc, x_q, x_kv, wq_t, wk_t, wv_t, wo_t,
          cosq, sinq, cosk, sink, emask_p, out_e, causal):
    from contextlib import ExitStack

    with ExitStack() as ctx:
        pool_xq = ctx.enter_context(tc.tile_pool(name="xq", bufs=1))
        pool_xkv = ctx.enter_context(tc.tile_pool(name="xkv", bufs=1))
        pool_attn = ctx.enter_context(tc.tile_pool(name="attnp", bufs=1))
        pool_q = ctx.enter_context(tc.tile_pool(name="qall", bufs=1))
        pool_mask = ctx.enter_context(tc.tile_pool(name="maskp", bufs=1))
        pool_exps = ctx.enter_context(tc.tile_pool(name="exps", bufs=3))
        pool_v = ctx.enter_context(tc.tile_pool(name="vsb", bufs=1))
        pool_kg = ctx.enter_context(tc.tile_pool(name="kg", bufs=2))
        pool_w = ctx.enter_context(tc.tile_pool(name="wrow", bufs=2))
        pool_wo = ctx.enter_context(tc.tile_pool(name="worow", bufs=6))
        pool_rot = ctx.enter_context(tc.tile_pool(name="rot", bufs=1))
        pool_tmp = ctx.enter_context(tc.tile_pool(name="tmp", bufs=1))
        pool_kv_out = ctx.enter_context(tc.tile_pool(name="kvout", bufs=1))
        pool_t1 = ctx.enter_context(tc.tile_pool(name="t1p", bufs=2))
        pool_tr = ctx.enter_context(tc.tile_pool(name="trp", bufs=2))
        pool_fold = ctx.enter_context(tc.tile_pool(name="fold", bufs=3))
        pool_recip = ctx.enter_context(tc.tile_pool(name="recip", bufs=2))
        pool_const = ctx.enter_context(tc.tile_pool(name="consts", bufs=1))
        pool_out = ctx.enter_context(tc.tile_pool(name="osb", bufs=2))
        pool_ps = ctx.enter_context(tc.tile_pool(name="psm", bufs=3, space="PSUM"))
        pool_pv = ctx.enter_context(tc.tile_pool(name="pspv", bufs=2, space="PSUM"))
        pool_dram = ctx.enter_context(tc.tile_pool(name="dram", bufs=1, space="DRAM"))

        # ---- early input streams (order = priority on the sync queue) --
        xkv_sb = pool_xkv.tile([P, CK, T], BF16, tag="xkv")
        wk_sb = [pool_w.tile([P, 8, KVW], BF16, tag="w", name=f"wk{i}") for i in range(2)]
        xkv_r = x_kv.rearrange("(ck p) t -> p ck t", p=P)
        wk_r = wk_t.rearrange("(ck p) n -> p ck n", p=P)
        for i in range(4):
            nc.sync.dma_start(xkv_sb[:, 4 * i : 4 * i + 4, :], xkv_r[:, 4 * i : 4 * i + 4, :])
            nc.sync.dma_start(
                wk_sb[i // 2][:, 4 * (i % 2) : 4 * (i % 2) + 4, :],
                wk_r[:, 4 * i : 4 * i + 4, :],
            )
        cosk_sb = pool_const.tile([P, T], F32, tag="cosk")
        nc.sync.dma_start(cosk_sb[:], cosk[:, :])
        sink_sb = pool_const.tile([P, T], F32, tag="sink")
        nc.sync.dma_start(sink_sb[:], sink[:, :])

        ag_in_k = pool_dram.tile([KVW, T], BF16)
        ag_out_k = pool_dram.tile([4 * KVW, T], BF16)
        ag_in_v = pool_dram.tile([KVW, T], BF16)
        ag_out_v = pool_dram.tile([4 * KVW, T], BF16)

        # ---- phase A1: K projection + RoPE(k) --------------------------
        kps = [pool_ps.tile([P, 2, T], F32, tag="ps", name=f"kps{i}") for i in range(2)]
        for ck in range(CK):
            first, last = ck == 0, ck == CK - 1
            wk_ap = wk_sb[ck // 8][:, ck % 8, :]
            for kvh in range(N_KV_HEADS):
                nc.tensor.matmul(
                    kps[kvh // 2][:, kvh % 2, :],
                    lhsT=wk_ap[:, kvh * HEAD_DIM : (kvh + 1) * HEAD_DIM],
                    rhs=xkv_sb[:, ck, :],
                    start=first,
                    stop=last,
                )
        # stream wv while K proj runs
        wv_sb = [pool_w.tile([P, 8, KVW], BF16, tag="w", name=f"wv{i}") for i in range(2)]
        wv_r = wv_t.rearrange("(ck p) n -> p ck n", p=P)
        for i in range(2):
            nc.sync.dma_start(wv_sb[i][:], wv_r[:, 8 * i : 8 * i + 8, :])

        kbf = pool_kv_out.tile([P, 4, T], BF16, tag="kvout")
        for kvh in range(N_KV_HEADS):
            _rope(nc, pool_rot, pool_tmp, kps[kvh // 2][:, kvh % 2, :],
                  cosk_sb, sink_sb, kbf[:, kvh, :])
        nc.gpsimd.dma_start(
            ag_in_k.rearrange("(kvh p) t -> p kvh t", p=P), kbf[:]
        )
        nc.gpsimd.collective_compute(
            "AllGather",
            mybir.AluOpType.bypass,
            replica_groups=GROUPS,
            ins=[ag_in_k[0:P, :].opt()],
            outs=[ag_out_k[0 : 4 * P, :].opt()],
        )

        # ---- phase A2: V projection (token-major) ----------------------
        vps = [pool_ps.tile([P, 2, T], F32, tag="ps", name=f"vps{i}") for i in range(2)]
        for ck in range(CK):
            first, last = ck == 0, ck == CK - 1
            wv_ap = wv_sb[ck // 8][:, ck % 8, :]
            for us in range(4):
                nc.tensor.matmul(
                    vps[us // 2][:, us % 2, :],
                    lhsT=xkv_sb[:, ck, us * P : (us + 1) * P],
                    rhs=wv_ap[:],
                    start=first,
                    stop=last,
                )
        vbf = pool_kv_out.tile([P, 4, T], BF16, tag="kvout")
        for us in range(4):
            nc.vector.tensor_copy(vbf[:, us, :], vps[us // 2][:, us % 2, :])
        nc.gpsimd.dma_start(
            ag_in_v.rearrange("(us p) t -> p us t", p=P), vbf[:]
        )
        # Gathers sliced so dependents unblock as early as possible: K heads
        # 0..3 individually (scores for group g need only K g); V in two
        # token-halves (PV chains consume half-A chunks first).  Stream
        # order: K0, Va, K1, Vb, K2, K3.
        def ag_k(g):
            nc.gpsimd.collective_compute(
                "AllGather",
                mybir.AluOpType.bypass,
                replica_groups=GROUPS,
                ins=[ag_in_k[g * P : (g + 1) * P, :].opt()],
                outs=[ag_out_k[g * 4 * P : (g + 1) * 4 * P, :].opt()],
            )
        def ag_v(half):
            nc.gpsimd.collective_compute(
                "AllGather",
                mybir.AluOpType.bypass,
                replica_groups=GROUPS,
                ins=[ag_in_v[half * 2 * P : (half + 1) * 2 * P, :].opt()],
                outs=[ag_out_v[half * 8 * P : (half + 1) * 8 * P, :].opt()],
            )
        ag_v(0)
        ag_k(1)
        ag_v(1)
        ag_k(2)
        ag_k(3)

        # ---- phase B: Q projection + RoPE ------------------------------
        # stream x_q, cos/sin(q), wq while A runs
        xq_sb = pool_xq.tile([P, CK, T], BF16, tag="xq")
        xq_r = x_q.rearrange("(ck p) t -> p ck t", p=P)
        for i in range(4):
            nc.sync.dma_start(xq_sb[:, 4 * i : 4 * i + 4, :], xq_r[:, 4 * i : 4 * i + 4, :])
        cosq_sb = pool_const.tile([P, T], F32, tag="cosq")
        nc.sync.dma_start(cosq_sb[:], cosq[:, :])
        sinq_sb = pool_const.tile([P, T], F32, tag="sinq")
        nc.sync.dma_start(sinq_sb[:], sinq[:, :])
        ones_sb = pool_const.tile([P, P], BF16, tag="ones")
        nc.vector.memset(ones_sb[:], 1.0)

        wq_r = wq_t.rearrange("(ck p) n -> p ck n", p=P)
        q_all = pool_q.tile([P, N_HEADS, T], BF16, tag="qall")
        for hg in range(4):
            cbase = hg * 4 * HEAD_DIM
            wq_sb = [pool_w.tile([P, 8, KVW], BF16, tag="w", name=f"wq{hg}_{i}") for i in range(2)]
            for i in range(2):
                nc.sync.dma_start(
                    wq_sb[i][:], wq_r[:, 8 * i : 8 * i + 8, cbase : cbase + 4 * HEAD_DIM]
                )
            qps = [pool_ps.tile([P, 2, T], F32, tag="ps", name=f"qps{hg}_{i}") for i in range(2)]
            for ck in range(CK):
                first, last = ck == 0, ck == CK - 1
                wq_ap = wq_sb[ck // 8][:, ck % 8, :]
                for hh in range(4):
                    nc.tensor.matmul(
                        qps[hh // 2][:, hh % 2, :],
                        lhsT=wq_ap[:, hh * HEAD_DIM : (hh + 1) * HEAD_DIM],
                        rhs=xq_sb[:, ck, :],
                        start=first,
                        stop=last,
                    )
            for hh in range(4):
                h = hg * 4 + hh
                _rope(nc, pool_rot, pool_tmp, qps[hh // 2][:, hh % 2, :],
                      cosq_sb, sinq_sb, q_all[:, h, :])

        # ---- mask tile (independent of AG; issued before k_g loads) ----
        # Packed to match exps exactly; dead regions hold 1.0 so a single
        # full-tile multiply applies the whole mask.
        NEXP = 12 if causal else UC
        em_sb = pool_mask.tile([P, NEXP, T], BF16, tag="maskp")
        nc.sync.dma_start(em_sb[:], emask_p.rearrange("(c p) t -> p c t", p=P))

        # ---- phase C: attention (pipelined two heads deep) -------------
        # ag_out_k block g: rows (j p) = K head g of shard j.
        # ag_out_v half a: rows (j us p), us in {0,1} = chunk 4j+us;
        #          half b: same with us in {2,3} = chunk 4j+2+us.
        attn_all = pool_attn.tile([P, N_HEADS, T], BF16, tag="attnp")

        # PV chunk order: half-a chunks first so the chain can start before
        # the second V gather lands.
        PV_ORDER = [c for j in range(4) for c in (4 * j, 4 * j + 1)] + [
            c for j in range(4) for c in (4 * j + 2, 4 * j + 3)
        ]

        def v_slot(c):
            # (tile_half, slot) for chunk c in the gathered V tiles
            j, us = divmod(c, 4)
            return us // 2, 2 * j + (us % 2)

        state = {}  # per-head tiles carried across the pipeline skew

        def emit_scores(h):
            g, hh = divmod(h, 4)
            if hh == 0:
                k_g = pool_kg.tile([P, 4, T], BF16, tag="kg", name=f"kg{g}")
                nc.sync.dma_start(
                    k_g[:],
                    ag_out_k[g * 4 * P : (g + 1) * 4 * P, :].rearrange(
                        "(j p) t -> p j t", p=P
                    ),
                )
                state["kg"] = k_g
                if g == 0:
                    v_ab = []
                    for half in range(2):
                        vt = pool_v.tile([P, 8, KVW], BF16, tag=f"v{half}")
                        nc.sync.dma_start(
                            vt[:],
                            ag_out_v[half * 8 * P : (half + 1) * 8 * P, :].rearrange(
                                "(c p) n -> p c n", p=P
                            ),
                        )
                        v_ab.append(vt)
                    state["v"] = v_ab
            k_g = state["kg"]
            exps = pool_exps.tile([P, NEXP, T], BF16, tag="exps", name=f"exps{h}")
            # chunks 0..7: full width
            for cp in range(4):
                pss = pool_ps.tile([P, 2, T], F32, tag="ps", name=f"ss{h}_{cp}")
                for half in range(2):
                    c = 2 * cp + half
                    j, r = divmod(c, 4)
                    nc.tensor.matmul(
                        pss[:, half, :],
                        lhsT=k_g[:, j, r * P : (r + 1) * P],
                        rhs=q_all[:, h, :],
                        start=True,
                        stop=True,
                    )
                nc.scalar.activation(
                    exps[:, 2 * cp : 2 * cp + 2, :],
                    pss[:],
                    mybir.ActivationFunctionType.Exp,
                )
            if causal:
                # chunks 8..15: late query half only, packed two chunks/row
                for qp in range(2):
                    psq = pool_ps.tile([P, 4, HT], F32, tag="ps", name=f"sq{h}_{qp}")
                    for s4 in range(4):
                        c = 8 + 4 * qp + s4
                        j, r = divmod(c, 4)
                        nc.tensor.matmul(
                            psq[:, s4, :],
                            lhsT=k_g[:, j, r * P : (r + 1) * P],
                            rhs=q_all[:, h, HT:T],
                            start=True,
                            stop=True,
                        )
                    nc.scalar.activation(
                        exps[:, 8 + 2 * qp : 10 + 2 * qp, :],
                        psq[:],
                        mybir.ActivationFunctionType.Exp,
                    )
                # single full-tile mask multiply (em holds 1.0 where dead)
                nc.vector.tensor_tensor(exps[:], exps[:], em_sb[:], MULT)
                fold = pool_fold.tile([P, T], BF16, tag="fold")
                with nc.allow_low_precision(reason="softmax denom bf16"):
                    t1 = pool_t1.tile([P, 4, T], BF16, tag="t1")
                    nc.vector.tensor_tensor(t1[:], exps[:, 0:4, :], exps[:, 4:8, :], ADD)
                    la = pool_tr.tile([P, 2, T], BF16, tag="tr", name=f"la{h}")
                    nc.gpsimd.tensor_tensor(la[:], exps[:, 8:10, :], exps[:, 10:12, :], ADD)
                    lb = pool_fold.tile([P, T], BF16, tag="lb")
                    nc.gpsimd.tensor_tensor(lb[:], la[:, 0, :], la[:, 1, :], ADD)
                    nc.vector.tensor_reduce(
                        fold[:],
                        t1[:].rearrange("p c t -> p t c"),
                        mybir.AxisListType.X,
                        ADD,
                    )
                    nc.gpsimd.tensor_tensor(
                        lb[:, 0:HT], lb[:, 0:HT], lb[:, HT:T], ADD
                    )
                    nc.vector.tensor_tensor(
                        fold[:, HT:T], fold[:, HT:T], lb[:, 0:HT], ADD
                    )
            else:
                for cp in range(4, 8):
                    pss = pool_ps.tile([P, 2, T], F32, tag="ps", name=f"ss{h}_{cp}")
                    for half in range(2):
                        c = 2 * cp + half
                        j, r = divmod(c, 4)
                        nc.tensor.matmul(
                            pss[:, half, :],
                            lhsT=k_g[:, j, r * P : (r + 1) * P],
                            rhs=q_all[:, h, :],
                            start=True,
                            stop=True,
                        )
                    nc.scalar.activation(
                        exps[:, 2 * cp : 2 * cp + 2, :],
                        pss[:],
                        mybir.ActivationFunctionType.Exp,
                    )
                nc.vector.tensor_tensor(exps[:], exps[:], em_sb[:], MULT)
                fold = pool_fold.tile([P, T], BF16, tag="fold")
                with nc.allow_low_precision(reason="softmax denom bf16"):
                    t1 = pool_t1.tile([P, 4, T], BF16, tag="t1")
                    nc.vector.tensor_tensor(t1[:], exps[:, 0:4, :], exps[:, 4:8, :], ADD)
                    nc.gpsimd.tensor_tensor(t1[:], t1[:], exps[:, 8:12, :], ADD)
                    nc.gpsimd.tensor_tensor(t1[:], t1[:], exps[:, 12:16, :], ADD)
                    nc.vector.tensor_reduce(
                        fold[:],
                        t1[:].rearrange("p c t -> p t c"),
                        mybir.AxisListType.X,
                        ADD,
                    )
            state[("exps", h)] = exps
            state[("fold", h)] = fold

        def emit_pv(h):
            g = h // 4
            exps = state.pop(("exps", h))
            fold = state.pop(("fold", h))
            v_ab = state["v"]
            pso = pool_pv.tile([P, T], F32, tag="pspv", name=f"o{h}")
            first_c = PV_ORDER[0]
            last_c = PV_ORDER[-1]
            for c in PV_ORDER:
                half, slot = v_slot(c)
                v_lhs = v_ab[half][:, slot, g * P : (g + 1) * P]
                if causal and c >= 8:
                    row = 8 + (c - 8) // 2
                    col = ((c - 8) % 2) * HT
                    nc.tensor.matmul(
                        pso[:, HT:T],
                        lhsT=v_lhs,
                        rhs=exps[:, row, col : col + HT],
                        start=False,
                        stop=(c == last_c),
                        skip_group_check=True,
                    )
                else:
                    nc.tensor.matmul(
                        pso[:],
                        lhsT=v_lhs,
                        rhs=exps[:, c, :],
                        start=(c == first_c),
                        stop=(c == last_c),
                        skip_group_check=True,
                    )
            psd = pool_ps.tile([P, 2, T], F32, tag="ps", name=f"d{h}")
            nc.tensor.matmul(
                psd[:, 0, :], lhsT=ones_sb[:], rhs=fold[:], start=True, stop=True
            )
            recip = pool_recip.tile([P, T], F32, tag="recip")
            nc.vector.reciprocal_approx_fast(recip[:], psd[:, 0, :])
            nc.vector.tensor_tensor(attn_all[:, h, :], pso[:], recip[:], MULT)

        for h in range(N_HEADS):
            emit_scores(h)
            if h >= 2:
                emit_pv(h - 2)
        emit_pv(N_HEADS - 2)
        emit_pv(N_HEADS - 1)

        # ---- phase D: output projection (wo streamed once) -------------
        wo_r = wo_t.rearrange("(j p) n -> p j n", p=P)
        out_r = out_e.rearrange("(t4 p) n -> p t4 n", p=P)
        for ec in range(4):
            psf = [
                pool_ps.tile([P, 2, 512], F32, tag="ps", name=f"f{ec}_{i}")
                for i in range(2)
            ]
            for jp in range(8):
                wo_sb = pool_wo.tile([P, 2, 512], BF16, tag="wo")
                nc.sync.dma_start(
                    wo_sb[:], wo_r[:, 2 * jp : 2 * jp + 2, ec * 512 : (ec + 1) * 512]
                )
                for ji in range(2):
                    j = 2 * jp + ji
                    first, last = j == 0, j == N_HEADS - 1
                    for t4 in range(4):
                        nc.tensor.matmul(
                            psf[t4 // 2][:, t4 % 2, :],
                            lhsT=attn_all[:, j, t4 * P : (t4 + 1) * P],
                            rhs=wo_sb[:, ji, :],
                            start=first,
                            stop=last,
                        )
            for t4 in range(4):
                osb = pool_out.tile([P, 512], F32, tag="o")
                nc.vector.tensor_copy(osb[:], psf[t4 // 2][:, t4 % 2, :])
                nc.scalar.dma_start(
                    out_r[:, t4, ec * 512 : (ec + 1) * 512], osb[:]
                )


_NC_CACHE = {}


def _get_graph(causal):
    if causal not in _NC_CACHE:
        _NC_CACHE[causal] = build_graph(causal)
    return _NC_CACHE[causal]


def _is_causal(mask):
    if mask.shape != (SEQLEN, SEQLEN):
        return False
    il = np.tril_indices(SEQLEN)
    if not np.all(mask[il] == 0.0):
        return False
    iu = np.triu_indices(SEQLEN, 1)
    return bool(np.all(mask[iu] < -1e8))


def _q_positions(j, causal):
    if causal:
        a, b = j, 7 - j
        return np.concatenate(
            [np.arange(a * HT, a * HT + HT), np.arange(b * HT, b * HT + HT)]
        )
    return np.arange(j * T, j * T + T)


def prep_in_maps(x, wq, wk, wv, wo, freqs_cos, freqs_sin, mask, causal=None):
    xf = np.asarray(x, dtype=np.float32).reshape(BATCH * SEQLEN, DIM)
    wq = np.asarray(wq, dtype=np.float32)
    wk = np.asarray(wk, dtype=np.float32)
    wv = np.asarray(wv, dtype=np.float32)
    wo = np.asarray(wo, dtype=np.float32)
    freqs_cos = np.asarray(freqs_cos, dtype=np.float32)
    freqs_sin = np.asarray(freqs_sin, dtype=np.float32)
    mask = np.asarray(mask, dtype=np.float32)
    if causal is None:
        causal = _is_causal(mask)

    perm = np.concatenate([np.arange(0, HEAD_DIM, 2), np.arange(1, HEAD_DIM, 2)])
    scale = 1.0 / math.sqrt(HEAD_DIM)
    wq_p = (wq.reshape(N_HEADS, HEAD_DIM, DIM)[:, perm, :] * scale).reshape(
        N_HEADS * HEAD_DIM, DIM
    )
    wk_p = wk.reshape(N_KV_HEADS, HEAD_DIM, DIM)[:, perm, :].reshape(KVW, DIM)
    wq_t = np.ascontiguousarray(wq_p.T).astype(BF)
    wk_t = np.ascontiguousarray(wk_p.T).astype(BF)
    wv_t = np.ascontiguousarray(wv.T).astype(BF)
    wo_t = np.ascontiguousarray(wo.T).astype(BF)
    emask_full = np.exp(mask)  # {0, 1} for causal/zero masks

    def rope_pair(pos_idx):
        cosb = freqs_cos[pos_idx].T  # [64, T]
        sinb = freqs_sin[pos_idx].T
        return (
            np.ascontiguousarray(np.concatenate([cosb, cosb], axis=0)),
            np.ascontiguousarray(np.concatenate([-sinb, sinb], axis=0)),
        )

    in_maps = []
    for i in range(N_CORES):
        b, j = divmod(i, 4)
        qpos = _q_positions(j, causal)
        kvpos = np.arange(j * T, j * T + T)
        cq, sq = rope_pair(qpos)
        ck_, sk_ = rope_pair(kvpos)
        if causal:
            qpos_e, qpos_l = qpos[:HT], qpos[HT:]
            # packed mask matching the exps layout; 1.0 in dead regions
            em_p = np.ones((12 * P, T), dtype=np.float32)
            for c in range(8):
                keys = np.arange(c * P, (c + 1) * P)
                em_p[c * P : (c + 1) * P, 0:HT] = emask_full[
                    np.ix_(qpos_e, keys)
                ].T
            for ii in range(4):
                keys_a = np.arange((8 + 2 * ii) * P, (9 + 2 * ii) * P)
                keys_b = np.arange((9 + 2 * ii) * P, (10 + 2 * ii) * P)
                em_p[(8 + ii) * P : (9 + ii) * P, 0:HT] = emask_full[
                    np.ix_(qpos_l, keys_a)
                ].T
                em_p[(8 + ii) * P : (9 + ii) * P, HT:T] = emask_full[
                    np.ix_(qpos_l, keys_b)
                ].T
            em_p = em_p.astype(BF)
        else:
            em_p = np.ascontiguousarray(emask_full[qpos, :].T).astype(BF)
        m = {
            "x_q": np.ascontiguousarray(xf[b * SEQLEN + qpos].T).astype(BF),
            "x_kv": np.ascontiguousarray(xf[b * SEQLEN + kvpos].T).astype(BF),
            "wq_t": wq_t,
            "wk_t": wk_t,
            "wv_t": wv_t,
            "wo_t": wo_t,
            "cosq": cq,
            "sinq": sq,
            "cosk": ck_,
            "sink": sk_,
            "emask_p": em_p,
        }
        in_maps.append(m)
    return in_maps, causal


def kernel(x, wq, wk, wv, wo, freqs_cos, freqs_sin, mask, start_pos):
    in_maps, causal = prep_in_maps(x, wq, wk, wv, wo, freqs_cos, freqs_sin, mask)
    nc = _get_graph(causal)
    res = run_bass_kernel_spmd(nc, in_maps, list(range(N_CORES)))

    out = np.empty((BATCH * SEQLEN, DIM), dtype=np.float32)
    for i in range(N_CORES):
        b, j = divmod(i, 4)
        qpos = _q_positions(j, causal)
        out[b * SEQLEN + qpos] = res.results[i]["out"]
    return out.reshape(BATCH, SEQLEN, DIM)


# revision 46
# speedup vs baseline: 1.1121x; 1.1121x over previous
"""Distributed Trainium2 attention kernel (8 NeuronCores), v2.

Reference computation (dense transformer attention block, prefill):
    q/k/v = x @ w{q,k,v}.T ; RoPE(q, k) ; GQA expand ; softmax(q k^T * scale + mask) v ; @ wo.T

Sharding: data-parallel over (batch x sequence): core i owns 512 tokens of
batch i//4.  Each core computes its k/v shard (contiguous 512-token block),
K and V are AllGathered in ONE collective within each batch's group of 4
cores (bf16), then each core runs attention for its 512 query tokens and
its rows of the output projection.

Query assignment (causal): core j of a batch group takes the 256-token
blocks {j, 7-j}.  Key chunks 0..7 are live for both halves (full-width
ops); chunks 8..15 only for the late half (half-width, packed two chunks
per exps row).

Schedule notes:
  - All input DMAs are batched (few large issues) and ordered so K proj
    starts ~5us in; gathers are sliced (K per head, V in token-halves)
    and interleaved K0,Va,K1,Vb,K2,K3 so attention group g unblocks as
    early as possible while later gathers hide behind compute.
  - exps stored packed [P, 12, 512]; one full-tile mask multiply (the
    packed mask holds 1.0 in dead regions).
  - softmax fold tree split between DVE and GpSimd; head loop software-
    pipelined two heads deep; the denominator matmul is emitted after
    the PV chain so the PE never waits on the fold tree.
  - PV chains consume half-A chunks first to start before Vb lands.
  - wo streamed once (not twice) in [P,2,512] tiles on the sync queue;
    final outputs written from the scalar engine's DMA queue.
"""

import math
import sys
import types

import numpy as np
import ml_dtypes

# ---------------------------------------------------------------------------
# antenv.axon_hooks shim: the container image's antenv package lacks
# axon_hooks; bass_utils imports it when BASS_TRACE is set.
if "antenv.axon_hooks" not in sys.modules:
    _hooks = types.ModuleType("antenv.axon_hooks")
    _hooks._hook = None
    _hooks.set_axon_ntff_profile_hook = lambda h: setattr(_hooks, "_hook", h)
    _hooks.get_axon_ntff_profile_hook = lambda: _hooks._hook
    sys.modules["antenv.axon_hooks"] = _hooks
    try:
        import antenv

        antenv.axon_hooks = _hooks
        from trn_agent_boot.trn_boot import _ntff_profile_via_ctypes

        _hooks.set_axon_ntff_profile_hook(
            _ntff_profile_via_ctypes("/opt/axon/libaxon_pjrt.so")
        )
    except Exception:
        pass

import concourse.bass as bass
import concourse.bacc as bacc
import concourse.mybir as mybir
import concourse.tile as tile
from concourse.bass_utils import run_bass_kernel_spmd

# Problem constants (hardcoded per spec nn_Attention_73040213836414).
DIM = 2048
N_HEADS = 16
N_KV_HEADS = 4
HEAD_DIM = 128
BATCH = 2
SEQLEN = 2048
N_CORES = 8
GROUPS = [[0, 1, 2, 3], [4, 5, 6, 7]]

P = 128
T = 512  # tokens per core
HT = T // 2  # 256, causal half-block
CK = DIM // P  # 16 contraction chunks
UC = SEQLEN // P  # 16 key chunks
KVW = N_KV_HEADS * HEAD_DIM  # 512

F32 = mybir.dt.float32
BF16 = mybir.dt.bfloat16
ADD = mybir.AluOpType.add
MULT = mybir.AluOpType.mult
BF = ml_dtypes.bfloat16


def build_graph(causal):
    nc = bacc.Bacc(
        "TRN2",
        target_bir_lowering=False,
        debug=False,
        enable_asserts=False,
        num_devices=N_CORES,
    )
    x_q = nc.dram_tensor("x_q", [DIM, T], BF16, kind="ExternalInput").ap()
    x_kv = nc.dram_tensor("x_kv", [DIM, T], BF16, kind="ExternalInput").ap()
    wq_t = nc.dram_tensor("wq_t", [DIM, N_HEADS * HEAD_DIM], BF16, kind="ExternalInput").ap()
    wk_t = nc.dram_tensor("wk_t", [DIM, KVW], BF16, kind="ExternalInput").ap()
    wv_t = nc.dram_tensor("wv_t", [DIM, KVW], BF16, kind="ExternalInput").ap()
    wo_t = nc.dram_tensor("wo_t", [DIM, DIM], BF16, kind="ExternalInput").ap()
    cosq = nc.dram_tensor("cosq", [P, T], F32, kind="ExternalInput").ap()
    sinq = nc.dram_tensor("sinq", [P, T], F32, kind="ExternalInput").ap()
    cosk = nc.dram_tensor("cosk", [P, T], F32, kind="ExternalInput").ap()
    sink = nc.dram_tensor("sink", [P, T], F32, kind="ExternalInput").ap()
    nexp = 12 if causal else UC
    emask_p = nc.dram_tensor("emask_p", [nexp * P, T], BF16, kind="ExternalInput").ap()
    out_e = nc.dram_tensor("out", [T, DIM], F32, kind="ExternalOutput").ap()

    with tile.TileContext(nc) as tc:
        _body(tc, nc, x_q, x_kv, wq_t, wk_t, wv_t, wo_t,
              cosq, sinq, cosk, sink, emask_p, out_e, causal)
    nc.compile()
    return nc


def _rope(nc, pool_rot, pool_tmp, psum_ap, cos_sb, sin_sb, out_ap):
    """out = psum*cos + rot_half(psum)*sin_signed, cast to out dtype."""
    rot = pool_rot.tile([P, T], F32, tag="rot")
    nc.vector.tensor_tensor(rot[0:64, :], psum_ap[64:128, :], sin_sb[0:64, :], MULT)
    nc.vector.tensor_tensor(rot[64:128, :], psum_ap[0:64, :], sin_sb[64:128, :], MULT)
    qc = pool_tmp.tile([P, T], F32, tag="tmp")
    nc.vector.tensor_tensor(qc[:], psum_ap[:], cos_sb[:], MULT)
    nc.vector.tensor_tensor(out_ap, qc[:], rot[:], ADD)


def _body(tc, nc, x_q, x_kv, wq_t, wk_t, wv_t, wo_t,
          cosq, sinq, cosk, sink, emask_p, out_e, causal):
    from contextlib import ExitStack

    with ExitStack() as ctx:
        pool_xq = ctx.enter_context(tc.tile_pool(name="xq", bufs=1))
        pool_xkv = ctx.enter_context(tc.tile_pool(name="xkv", bufs=1))
        pool_attn = ctx.enter_context(tc.tile_pool(name="attnp", bufs=1))
        pool_q = ctx.enter_context(tc.tile_pool(name="qall", bufs=1))
        pool_mask = ctx.enter_context(tc.tile_pool(name="maskp", bufs=1))
        pool_exps = ctx.enter_context(tc.tile_pool(name="exps", bufs=3))
        pool_v = ctx.enter_context(tc.tile_pool(name="vsb", bufs=1))
        pool_kg = ctx.enter_context(tc.tile_pool(name="kg", bufs=2))
        pool_w = ctx.enter_context(tc.tile_pool(name="wrow", bufs=2))
        pool_wo = ctx.enter_context(tc.tile_pool(name="worow", bufs=6))
        pool_rot = ctx.enter_context(tc.tile_pool(name="rot", bufs=1))
        pool_tmp = ctx.enter_context(tc.tile_pool(name="tmp", bufs=1))
        pool_kv_out = ctx.enter_context(tc.tile_pool(name="kvout", bufs=1))
        pool_t1 = ctx.enter_context(tc.tile_pool(name="t1p", bufs=2))
        pool_tr = ctx.enter_context(tc.tile_pool(name="trp", bufs=2))
        pool_fold = ctx.enter_context(tc.tile_pool(name="fold", bufs=3))
        pool_recip = ctx.enter_context(tc.tile_pool(name="recip", bufs=2))
        pool_const = ctx.enter_context(tc.tile_pool(name="consts", bufs=1))
        pool_out = ctx.enter_context(tc.tile_pool(name="osb", bufs=2))
        pool_ps = ctx.enter_context(tc.tile_pool(name="psm", bufs=3, space="PSUM"))
        pool_pv = ctx.enter_context(tc.tile_pool(name="pspv", bufs=2, space="PSUM"))
        pool_dram = ctx.enter_context(tc.tile_pool(name="dram", bufs=1, space="DRAM"))

        # ---- early input streams (order = priority on the sync queue) --
        xkv_sb = pool_xkv.tile([P, CK, T], BF16, tag="xkv")
        wk_sb = [pool_w.tile([P, 8, KVW], BF16, tag="w", name=f"wk{i}") for i in range(2)]
        xkv_r = x_kv.rearrange("(ck p) t -> p ck t", p=P)
        wk_r = wk_t.rearrange("(ck p) n -> p ck n", p=P)
        for i in range(4):
            nc.sync.dma_start(xkv_sb[:, 4 * i : 4 * i + 4, :], xkv_r[:, 4 * i : 4 * i + 4, :])
            nc.sync.dma_start(
                wk_sb[i // 2][:, 4 * (i % 2) : 4 * (i % 2) + 4, :],
                wk_r[:, 4 * i : 4 * i + 4, :],
            )
        cosk_sb = pool_const.tile([P, T], F32, tag="cosk")
        nc.sync.dma_start(cosk_sb[:], cosk[:, :])
        sink_sb = pool_const.tile([P, T], F32, tag="sink")
        nc.sync.dma_start(sink_sb[:], sink[:, :])

        ag_in_k = pool_dram.tile([KVW, T], BF16)
        ag_out_k = pool_dram.tile([4 * KVW, T], BF16)
        ag_in_v = pool_dram.tile([KVW, T], BF16)
        ag_out_v = pool_dram.tile([4 * KVW, T], BF16)

        # ---- phase A1: K projection + RoPE(k) --------------------------
        kps = [pool_ps.tile([P, 2, T], F32, tag="ps", name=f"kps{i}") for i in range(2)]
        for ck in range(CK):
            first, last = ck == 0, ck == CK - 1
            wk_ap = wk_sb[ck // 8][:, ck % 8, :]
            for kvh in range(N_KV_HEADS):
                nc.tensor.matmul(
                    kps[kvh // 2][:, kvh % 2, :],
                    lhsT=wk_ap[:, kvh * HEAD_DIM : (kvh + 1) * HEAD_DIM],
                    rhs=xkv_sb[:, ck, :],
                    start=first,
                    stop=last,
                )
        # stream wv while K proj runs
        wv_sb = [pool_w.tile([P, 8, KVW], BF16, tag="w", name=f"wv{i}") for i in range(2)]
        wv_r = wv_t.rearrange("(ck p) n -> p ck n", p=P)
        for i in range(2):
            nc.sync.dma_start(wv_sb[i][:], wv_r[:, 8 * i : 8 * i + 8, :])

        kbf = pool_kv_out.tile([P, 4, T], BF16, tag="kvout")
        for kvh in range(N_KV_HEADS):
            _rope(nc, pool_rot, pool_tmp, kps[kvh // 2][:, kvh % 2, :],
                  cosk_sb, sink_sb, kbf[:, kvh, :])
        nc.gpsimd.dma_start(
            ag_in_k.rearrange("(kvh p) t -> p kvh t", p=P), kbf[:]
        )
        nc.gpsimd.collective_compute(
            "AllGather",
            mybir.AluOpType.bypass,
            replica_groups=GROUPS,
            ins=[ag_in_k[0:P, :].opt()],
            outs=[ag_out_k[0 : 4 * P, :].opt()],
        )

        # ---- phase A2: V projection (token-major) ----------------------
        vps = [pool_ps.tile([P, 2, T], F32, tag="ps", name=f"vps{i}") for i in range(2)]
        for ck in range(CK):
            first, last = ck == 0, ck == CK - 1
            wv_ap = wv_sb[ck // 8][:, ck % 8, :]
            for us in range(4):
                nc.tensor.matmul(
                    vps[us // 2][:, us % 2, :],
                    lhsT=xkv_sb[:, ck, us * P : (us + 1) * P],
                    rhs=wv_ap[:],
                    start=first,
                    stop=last,
                )
        vbf = pool_kv_out.tile([P, 4, T], BF16, tag="kvout")
        for us in range(4):
            nc.vector.tensor_copy(vbf[:, us, :], vps[us // 2][:, us % 2, :])
        nc.gpsimd.dma_start(
            ag_in_v.rearrange("(us p) t -> p us t", p=P), vbf[:]
        )
        # Gathers sliced so dependents unblock as early as possible: K heads
        # 0..3 individually (scores for group g need only K g); V in two
        # token-halves (PV chains consume half-A chunks first).  Stream
        # order: K0, Va, K1, Vb, K2, K3.
        def ag_k(g):
            nc.gpsimd.collective_compute(
                "AllGather",
                mybir.AluOpType.bypass,
                replica_groups=GROUPS,
                ins=[ag_in_k[g * P : (g + 1) * P, :].opt()],
                outs=[ag_out_k[g * 4 * P : (g + 1) * 4 * P, :].opt()],
            )
        def ag_v(half):
            nc.gpsimd.collective_compute(
                "AllGather",
                mybir.AluOpType.bypass,
                replica_groups=GROUPS,
                ins=[ag_in_v[half * 2 * P : (half + 1) * 2 * P, :].opt()],
                outs=[ag_out_v[half * 8 * P : (half + 1) * 8 * P, :].opt()],
            )
        ag_v(0)
        ag_k(1)
        ag_v(1)
        ag_k(2)
        ag_k(3)

        # ---- phase B: Q projection + RoPE ------------------------------
        # stream x_q, cos/sin(q), wq while A runs
        xq_sb = pool_xq.tile([P, CK, T], BF16, tag="xq")
        xq_r = x_q.rearrange("(ck p) t -> p ck t", p=P)
        for i in range(4):
            nc.sync.dma_start(xq_sb[:, 4 * i : 4 * i + 4, :], xq_r[:, 4 * i : 4 * i + 4, :])
        cosq_sb = pool_const.tile([P, T], F32, tag="cosq")
        nc.sync.dma_start(cosq_sb[:], cosq[:, :])
        sinq_sb = pool_const.tile([P, T], F32, tag="sinq")
        nc.sync.dma_start(sinq_sb[:], sinq[:, :])
        ones_sb = pool_const.tile([P, P], BF16, tag="ones")
        nc.vector.memset(ones_sb[:], 1.0)

        wq_r = wq_t.rearrange("(ck p) n -> p ck n", p=P)
        q_all = pool_q.tile([P, N_HEADS, T], BF16, tag="qall")
        for hg in range(4):
            cbase = hg * 4 * HEAD_DIM
            wq_sb = [pool_w.tile([P, 8, KVW], BF16, tag="w", name=f"wq{hg}_{i}") for i in range(2)]
            for i in range(2):
                nc.sync.dma_start(
                    wq_sb[i][:], wq_r[:, 8 * i : 8 * i + 8, cbase : cbase + 4 * HEAD_DIM]
                )
            qps = [pool_ps.tile([P, 2, T], F32, tag="ps", name=f"qps{hg}_{i}") for i in range(2)]
            for ck in range(CK):
                first, last = ck == 0, ck == CK - 1
                wq_ap = wq_sb[ck // 8][:, ck % 8, :]
                for hh in range(4):
                    nc.tensor.matmul(
                        qps[hh // 2][:, hh % 2, :],
                        lhsT=wq_ap[:, hh * HEAD_DIM : (hh + 1) * HEAD_DIM],
                        rhs=xq_sb[:, ck, :],
                        start=first,
                        stop=last,
                    )
            for hh in range(4):
                h = hg * 4 + hh
                _rope(nc, pool_rot, pool_tmp, qps[hh // 2][:, hh % 2, :],
                      cosq_sb, sinq_sb, q_all[:, h, :])

        # ---- mask tile (independent of AG; issued before k_g loads) ----
        # Packed to match exps exactly; dead regions hold 1.0 so a single
        # full-tile multiply applies the whole mask.
        NEXP = 12 if causal else UC
        em_sb = pool_mask.tile([P, NEXP, T], BF16, tag="maskp")
        nc.sync.dma_start(em_sb[:], emask_p.rearrange("(c p) t -> p c t", p=P))

        # ---- phase C: attention (pipelined two heads deep) -------------
        # ag_out_k block g: rows (j p) = K head g of shard j.
        # ag_out_v half a: rows (j us p), us in {0,1} = chunk 4j+us;
        #          half b: same with us in {2,3} = chunk 4j+2+us.
        attn_all = pool_attn.tile([P, N_HEADS, T], BF16, tag="attnp")

        # PV chunk order: half-a chunks first so the chain can start before
        # the second V gather lands.
        PV_ORDER = [c for j in range(4) for c in (4 * j, 4 * j + 1)] + [
            c for j in range(4) for c in (4 * j + 2, 4 * j + 3)
        ]

        def v_slot(c):
            # (tile_half, slot) for chunk c in the gathered V tiles
            j, us = divmod(c, 4)
            return us // 2, 2 * j + (us % 2)

        state = {}  # per-head tiles carried across the pipeline skew

        def emit_scores(h):
            g, hh = divmod(h, 4)
            if hh == 0:
                k_g = pool_kg.tile([P, 4, T], BF16, tag="kg", name=f"kg{g}")
                nc.sync.dma_start(
                    k_g[:],
                    ag_out_k[g * 4 * P : (g + 1) * 4 * P, :].rearrange(
                        "(j p) t -> p j t", p=P
                    ),
                )
                state["kg"] = k_g
                if g == 0:
                    v_ab = []
                    for half in range(2):
                        vt = pool_v.tile([P, 8, KVW], BF16, tag=f"v{half}")
                        nc.sync.dma_start(
                            vt[:],
                            ag_out_v[half * 8 * P : (half + 1) * 8 * P, :].rearrange(
                                "(c p) n -> p c n", p=P
                            ),
                        )
                        v_ab.append(vt)
                    state["v"] = v_ab
            k_g = state["kg"]
            exps = pool_exps.tile([P, NEXP, T], BF16, tag="exps", name=f"exps{h}")
            # chunks 0..7: full width
            for cp in range(4):
                pss = pool_ps.tile([P, 2, T], F32, tag="ps", name=f"ss{h}_{cp}")
                for half in range(2):
                    c = 2 * cp + half
                    j, r = divmod(c, 4)
                    nc.tensor.matmul(
                        pss[:, half, :],
                        lhsT=k_g[:, j, r * P : (r + 1) * P],
                        rhs=q_all[:, h, :],
                        start=True,
                        stop=True,
                    )
                nc.scalar.activation(
                    exps[:, 2 * cp : 2 * cp + 2, :],
                    pss[:],
                    mybir.ActivationFunctionType.Exp,
                )
            if causal:
                # chunks 8..15: late query half only, packed two chunks/row
                for qp in range(2):
                    psq = pool_ps.tile([P, 4, HT], F32, tag="ps", name=f"sq{h}_{qp}")
                    for s4 in range(4):
                        c = 8 + 4 * qp + s4
                        j, r = divmod(c, 4)
                        nc.tensor.matmul(
                            psq[:, s4, :],
                            lhsT=k_g[:, j, r * P : (r + 1) * P],
                            rhs=q_all[:, h, HT:T],
                            start=True,
                            stop=True,
                        )
                    nc.scalar.activation(
                        exps[:, 8 + 2 * qp : 10 + 2 * qp, :],
                        psq[:],
                        mybir.ActivationFunctionType.Exp,
                    )
                # single full-tile mask multiply (em holds 1.0 where dead)
                nc.vector.tensor_tensor(exps[:], exps[:], em_sb[:], MULT)
                fold = pool_fold.tile([P, T], BF16, tag="fold")
                with nc.allow_low_precision(reason="softmax denom bf16"):
                    t1 = pool_t1.tile([P, 4, T], BF16, tag="t1")
                    nc.vector.tensor_tensor(t1[:], exps[:, 0:4, :], exps[:, 4:8, :], ADD)
                    la = pool_tr.tile([P, 2, T], BF16, tag="tr", name=f"la{h}")
                    nc.gpsimd.tensor_tensor(la[:], exps[:, 8:10, :], exps[:, 10:12, :], ADD)
                    lb = pool_fold.tile([P, T], BF16, tag="lb")
                    nc.gpsimd.tensor_tensor(lb[:], la[:, 0, :], la[:, 1, :], ADD)
                    t2 = pool_tr.tile([P, 2, T], BF16, tag="tr", name=f"t2{h}")
                    nc.vector.tensor_tensor(t2[:], t1[:, 0:2, :], t1[:, 2:4, :], ADD)
                    nc.vector.tensor_tensor(fold[:], t2[:, 0, :], t2[:, 1, :], ADD)
                    nc.gpsimd.tensor_tensor(
                        lb[:, 0:HT], lb[:, 0:HT], lb[:, HT:T], ADD
                    )
                    nc.vector.tensor_tensor(
                        fold[:, HT:T], fold[:, HT:T], lb[:, 0:HT], ADD
                    )
            else:
                for cp in range(4, 8):
                    pss = pool_ps.tile([P, 2, T], F32, tag="ps", name=f"ss{h}_{cp}")
                    for half in range(2):
                        c = 2 * cp + half
                        j, r = divmod(c, 4)
                        nc.tensor.matmul(
                            pss[:, half, :],
                            lhsT=k_g[:, j, r * P : (r + 1) * P],
                            rhs=q_all[:, h, :],
                            start=True,
                            stop=True,
                        )
                    nc.scalar.activation(
                        exps[:, 2 * cp : 2 * cp + 2, :],
                        pss[:],
                        mybir.ActivationFunctionType.Exp,
                    )
                nc.vector.tensor_tensor(exps[:], exps[:], em_sb[:], MULT)
                fold = pool_fold.tile([P, T], BF16, tag="fold")
                with nc.allow_low_precision(reason="softmax denom bf16"):
                    t1 = pool_t1.tile([P, 4, T], BF16, tag="t1")
                    nc.vector.tensor_tensor(t1[:], exps[:, 0:4, :], exps[:, 4:8, :], ADD)
                    nc.gpsimd.tensor_tensor(t1[:], t1[:], exps[:, 8:12, :], ADD)
                    nc.gpsimd.tensor_tensor(t1[:], t1[:], exps[:, 12:16, :], ADD)
                    t2 = pool_tr.tile([P, 2, T], BF16, tag="tr", name=f"t2{h}")
                    nc.gpsimd.tensor_tensor(t2[:], t1[:, 0:2, :], t1[:, 2:4, :], ADD)
                    nc.vector.tensor_tensor(fold[:], t2[:, 0, :], t2[:, 1, :], ADD)
            state[("exps", h)] = exps
            state[("fold", h)] = fold

        def emit_pv(h):
            g = h // 4
            exps = state.pop(("exps", h))
            fold = state.pop(("fold", h))
            v_ab = state["v"]
            pso = pool_pv.tile([P, T], F32, tag="pspv", name=f"o{h}")
            first_c = PV_ORDER[0]
            last_c = PV_ORDER[-1]
            for c in PV_ORDER:
                half, slot = v_slot(c)
                v_lhs = v_ab[half][:, slot, g * P : (g + 1) * P]
                if causal and c >= 8:
                    row = 8 + (c - 8) // 2
                    col = ((c - 8) % 2) * HT
                    nc.tensor.matmul(
                        pso[:, HT:T],
                        lhsT=v_lhs,
                        rhs=exps[:, row, col : col + HT],
                        start=False,
                        stop=(c == last_c),
                        skip_group_check=True,
                    )
                else:
                    nc.tensor.matmul(
                        pso[:],
                        lhsT=v_lhs,
                        rhs=exps[:, c, :],
                        start=(c == first_c),
                        stop=(c == last_c),
                        skip_group_check=True,
                    )
            psd = pool_ps.tile([P, 2, T], F32, tag="ps", name=f"d{h}")
            nc.tensor.matmul(
                psd[:, 0, :], lhsT=ones_sb[:], rhs=fold[:], start=True, stop=True
            )
            recip = pool_recip.tile([P, T], F32, tag="recip")
            nc.vector.reciprocal_approx_fast(recip[:], psd[:, 0, :])
            nc.vector.tensor_tensor(attn_all[:, h, :], pso[:], recip[:], MULT)

        for h in range(N_HEADS):
            emit_scores(h)
            if h >= 2:
                emit_pv(h - 2)
        emit_pv(N_HEADS - 2)
        emit_pv(N_HEADS - 1)

        # ---- phase D: output projection (wo streamed once) -------------
        wo_r = wo_t.rearrange("(j p) n -> p j n", p=P)
        out_r = out_e.rearrange("(t4 p) n -> p t4 n", p=P)
        for ec in range(4):
            psf = [
                pool_ps.tile([P, 2, 512], F32, tag="ps", name=f"f{ec}_{i}")
                for i in range(2)
            ]
            for jp in range(8):
                wo_sb = pool_wo.tile([P, 2, 512], BF16, tag="wo")
                nc.sync.dma_start(
                    wo_sb[:], wo_r[:, 2 * jp : 2 * jp + 2, ec * 512 : (ec + 1) * 512]
                )
                for ji in range(2):
                    j = 2 * jp + ji
                    first, last = j == 0, j == N_HEADS - 1
                    for t4 in range(4):
                        nc.tensor.matmul(
                            psf[t4 // 2][:, t4 % 2, :],
                            lhsT=attn_all[:, j, t4 * P : (t4 + 1) * P],
                            rhs=wo_sb[:, ji, :],
                            start=first,
                            stop=last,
                        )
            for t4 in range(4):
                osb = pool_out.tile([P, 512], F32, tag="o")
                nc.vector.tensor_copy(osb[:], psf[t4 // 2][:, t4 % 2, :])
                nc.scalar.dma_start(
                    out_r[:, t4, ec * 512 : (ec + 1) * 512], osb[:]
                )


_NC_CACHE = {}


def _get_graph(causal):
    if causal not in _NC_CACHE:
        _NC_CACHE[causal] = build_graph(causal)
    return _NC_CACHE[causal]


def _is_causal(mask):
    if mask.shape != (SEQLEN, SEQLEN):
        return False
    il = np.tril_indices(SEQLEN)
    if not np.all(mask[il] == 0.0):
        return False
    iu = np.triu_indices(SEQLEN, 1)
    return bool(np.all(mask[iu] < -1e8))


def _q_positions(j, causal):
    if causal:
        a, b = j, 7 - j
        return np.concatenate(
            [np.arange(a * HT, a * HT + HT), np.arange(b * HT, b * HT + HT)]
        )
    return np.arange(j * T, j * T + T)


def prep_in_maps(x, wq, wk, wv, wo, freqs_cos, freqs_sin, mask, causal=None):
    xf = np.asarray(x, dtype=np.float32).reshape(BATCH * SEQLEN, DIM)
    wq = np.asarray(wq, dtype=np.float32)
    wk = np.asarray(wk, dtype=np.float32)
    wv = np.asarray(wv, dtype=np.float32)
    wo = np.asarray(wo, dtype=np.float32)
    freqs_cos = np.asarray(freqs_cos, dtype=np.float32)
    freqs_sin = np.asarray(freqs_sin, dtype=np.float32)
    mask = np.asarray(mask, dtype=np.float32)
    if causal is None:
        causal = _is_causal(mask)

    perm = np.concatenate([np.arange(0, HEAD_DIM, 2), np.arange(1, HEAD_DIM, 2)])
    scale = 1.0 / math.sqrt(HEAD_DIM)
    wq_p = (wq.reshape(N_HEADS, HEAD_DIM, DIM)[:, perm, :] * scale).reshape(
        N_HEADS * HEAD_DIM, DIM
    )
    wk_p = wk.reshape(N_KV_HEADS, HEAD_DIM, DIM)[:, perm, :].reshape(KVW, DIM)
    wq_t = np.ascontiguousarray(wq_p.T).astype(BF)
    wk_t = np.ascontiguousarray(wk_p.T).astype(BF)
    wv_t = np.ascontiguousarray(wv.T).astype(BF)
    wo_t = np.ascontiguousarray(wo.T).astype(BF)
    emask_full = np.exp(mask)  # {0, 1} for causal/zero masks

    def rope_pair(pos_idx):
        cosb = freqs_cos[pos_idx].T  # [64, T]
        sinb = freqs_sin[pos_idx].T
        return (
            np.ascontiguousarray(np.concatenate([cosb, cosb], axis=0)),
            np.ascontiguousarray(np.concatenate([-sinb, sinb], axis=0)),
        )

    in_maps = []
    for i in range(N_CORES):
        b, j = divmod(i, 4)
        qpos = _q_positions(j, causal)
        kvpos = np.arange(j * T, j * T + T)
        cq, sq = rope_pair(qpos)
        ck_, sk_ = rope_pair(kvpos)
        if causal:
            qpos_e, qpos_l = qpos[:HT], qpos[HT:]
            # packed mask matching the exps layout; 1.0 in dead regions
            em_p = np.ones((12 * P, T), dtype=np.float32)
            for c in range(8):
                keys = np.arange(c * P, (c + 1) * P)
                em_p[c * P : (c + 1) * P, 0:HT] = emask_full[
                    np.ix_(qpos_e, keys)
                ].T
            for ii in range(4):
                keys_a = np.arange((8 + 2 * ii) * P, (9 + 2 * ii) * P)
                keys_b = np.arange((9 + 2 * ii) * P, (10 + 2 * ii) * P)
                em_p[(8 + ii) * P : (9 + ii) * P, 0:HT] = emask_full[
                    np.ix_(qpos_l, keys_a)
                ].T
                em_p[(8 + ii) * P : (9 + ii) * P, HT:T] = emask_full[
                    np.ix_(qpos_l, keys_b)
                ].T
            em_p = em_p.astype(BF)
        else:
            em_p = np.ascontiguousarray(emask_full[qpos, :].T).astype(BF)
        m = {
            "x_q": np.ascontiguousarray(xf[b * SEQLEN + qpos].T).astype(BF),
            "x_kv": np.ascontiguousarray(xf[b * SEQLEN + kvpos].T).astype(BF),
            "wq_t": wq_t,
            "wk_t": wk_t,
            "wv_t": wv_t,
            "wo_t": wo_t,
            "cosq": cq,
            "sinq": sq,
            "cosk": ck_,
            "sink": sk_,
            "emask_p": em_p,
        }
        in_maps.append(m)
    return in_maps, causal


def kernel(x, wq, wk, wv, wo, freqs_cos, freqs_sin, mask, start_pos):
    in_maps, causal = prep_in_maps(x, wq, wk, wv, wo, freqs_cos, freqs_sin, mask)
    nc = _get_graph(causal)
    res = run_bass_kernel_spmd(nc, in_maps, list(range(N_CORES)))

    out = np.empty((BATCH * SEQLEN, DIM), dtype=np.float32)
    for i in range(N_CORES):
        b, j = divmod(i, 4)
        qpos = _q_positions(j, causal)
        out[b * SEQLEN + qpos] = res.results[i]["out"]
    return out.reshape(BATCH, SEQLEN, DIM)
